# revision 27
# baseline (speedup 1.0000x reference)
"""Trainium2 Bass kernel for a 6-layer transformer decoder (nn_Decoder).

v2 sharding: data-parallel over (batch, q-tiles): core c owns 4 q-tiles of
batch c//2 — even cores own global q-tiles {0,3,4,7}, odd cores {1,2,5,6}
(balanced causal load).  K/V projections are split across the pair: each
core projects K^T/V^T only for its OWN tiles (self-attn: from its local
u1T rows; cross-attn: from its half of enc), then K/V are AllGather'd
within the pair, chunked by head groups so the collective pipelines behind
Q-projection and the peer phase's compute.  Self-attention score work is
causally pruned: per t-chunk only the (uniform-across-cores) q-column
suffix that any core needs is computed; per-core mask data kills the
overlap slop.

All matmuls run in fp16 (fp32 PSUM accumulate).  LayerNorm gain/bias are
folded into the consuming weights on the host; per-feature biases are
applied as per-partition ACT biases (feature-major outputs) or rank-1
matmul rows (row-major outputs).  Softmax uses the transposed-logits
layout [t, q]: exp via ACT, denominator via a ones-matrix matmul
(partition-broadcast row sums), normalization folded into the PSUM->SBUF
copy of att@V.
"""

import numpy as np

import concourse.bacc as bacc
import concourse.mybir as mybir
from concourse.tile import TileContext
from concourse.bass_utils import run_bass_kernel_spmd
from concourse.masks import make_identity

F32, F16 = mybir.dt.float32, mybir.dt.float16
AF = mybir.ActivationFunctionType

H, M, DK, FF, L = 8, 1024, 128, 4096, 6
B, Q, T = 4, 1024, 1024
EPS = 1e-5
SC = float(DK) ** -0.5
NCORE = 8
RW = 512                    # rows owned per core
NQT = RW // 128             # q-tiles per core
NMC = M // 128              # m-chunks
NFC = FF // 128             # f-chunks
NTC = T // 128              # t-chunks
GROUPS = [[0, 1], [2, 3], [4, 5], [6, 7]]
MASK_CLIP = -30000.0        # representable in fp16; exp() -> 0
GH = 4                      # heads per AllGather chunk
NGS = H // GH

TILES_EVEN = [0, 3, 4, 7]   # global q-tile ids owned by group rank 0
TILES_ODD = [1, 2, 5, 6]
# global t-chunk -> (source rank, slot) in the gathered K/V
OWN = {}
for _sl, _g in enumerate(TILES_EVEN):
    OWN[_g] = (0, _sl)
for _sl, _g in enumerate(TILES_ODD):
    OWN[_g] = (1, _sl)

_CACHE = {}
ABLATE = set()   # dev: timeline-sim ablation flags


# --------------------------------------------------------------------------
# device program
# --------------------------------------------------------------------------

def build_decoder(nlayers, s_suf, s_regions, c_suf, c_regions,
                  sm_cols, cm_cols, local_cc=False):
    """s_suf/c_suf: per-t-chunk computed q-column suffix width (uniform
    across cores; mask data covers per-core slop).  *_regions: list of
    (tc, qlo, qw, col_off) DVE mask-add regions.  local_cc=True replaces
    AllGathers with local DMA copies (for TimelineSim)."""
    nc = bacc.Bacc(None)

    h0 = nc.dram_tensor("h0", [NQT, 128, M], F32, kind="ExternalInput")
    encT = nc.dram_tensor("encT", [NMC, 128, RW], F16, kind="ExternalInput")
    oscale = nc.dram_tensor("oscale", [NQT, 128, 1], F32, kind="ExternalInput")
    smsk = (nc.dram_tensor("smsk", [128, sm_cols], F16, kind="ExternalInput")
            if sm_cols else None)
    cmsk = (nc.dram_tensor("cmsk", [128, cm_cols], F16, kind="ExternalInput")
            if cm_cols else None)

    # per-head projection weights, partition-major: [l, h, 128, mc, DK]
    sqw = nc.dram_tensor("sqw", [nlayers, H, 128, NMC, DK], F16, kind="ExternalInput")
    skw = nc.dram_tensor("skw", [nlayers, H, 128, NMC, DK], F16, kind="ExternalInput")
    svw = nc.dram_tensor("svw", [nlayers, H, 128, NMC, DK], F16, kind="ExternalInput")
    swo = nc.dram_tensor("swo", [nlayers, 2, 128, H, 512], F16, kind="ExternalInput")
    qb1 = nc.dram_tensor("qb1", [nlayers, 128, H], F32, kind="ExternalInput")
    kb1 = nc.dram_tensor("kb1", [nlayers, 128, H], F32, kind="ExternalInput")
    vb1 = nc.dram_tensor("vb1", [nlayers, 128, H], F32, kind="ExternalInput")

    cqw = nc.dram_tensor("cqw", [nlayers, H, 128, NMC, DK], F16, kind="ExternalInput")
    ckw = nc.dram_tensor("ckw", [nlayers, H, 128, NMC, DK], F16, kind="ExternalInput")
    cvw = nc.dram_tensor("cvw", [nlayers, H, 128, NMC, DK], F16, kind="ExternalInput")
    cwo = nc.dram_tensor("cwo", [nlayers, 2, 128, H, 512], F16, kind="ExternalInput")
    qb2 = nc.dram_tensor("qb2", [nlayers, 128, H], F32, kind="ExternalInput")

    fw1 = nc.dram_tensor("fw1", [nlayers, 8, 128, NMC, 512], F16, kind="ExternalInput")
    fb1 = nc.dram_tensor("fb1", [nlayers, 128, NFC], F32, kind="ExternalInput")
    fw2 = nc.dram_tensor("fw2", [nlayers, 2, 8, 128, 4, 512], F16, kind="ExternalInput")
    fb2 = nc.dram_tensor("fb2", [nlayers, 1, M], F16, kind="ExternalInput")

    hout = nc.dram_tensor("hout", [NQT, 128, M], F32, kind="ExternalOutput")

    # gather buffers: ccin [H, 128, 2(k/v), RW]; ccout adds [2(src), GH, ...]
    cis = [nc.dram_tensor(f"cis{l}", [H, 128, 2, RW], F16)
           for l in range(nlayers)]
    cos = [nc.dram_tensor(f"cos{l}", [NGS, 2, GH, 128, 2, RW], F16)
           for l in range(nlayers)]
    cic = [nc.dram_tensor(f"cic{l}", [H, 128, 2, RW], F16)
           for l in range(nlayers)]
    coc = [nc.dram_tensor(f"coc{l}", [NGS, 2, GH, 128, 2, RW], F16)
           for l in range(nlayers)]

    with TileContext(nc) as tc:
        with (
            tc.tile_pool(name="cst", bufs=1) as cst,
            tc.tile_pool(name="hp", bufs=1) as hp,
            tc.tile_pool(name="ep", bufs=1) as ep,
            tc.tile_pool(name="lnp", bufs=2) as lnp,
            tc.tile_pool(name="utp", bufs=2) as utp,
            tc.tile_pool(name="qp", bufs=1) as qp,
            tc.tile_pool(name="whp", bufs=2) as whp,
            tc.tile_pool(name="kvp", bufs=3) as kvp,
            tc.tile_pool(name="kgp", bufs=2) as kgp,
            tc.tile_pool(name="vp", bufs=2) as vp,
            tc.tile_pool(name="exp2", bufs=2) as exp2,
            tc.tile_pool(name="big1", bufs=1) as big1,
            tc.tile_pool(name="wop", bufs=1) as wop,
            tc.tile_pool(name="f1p", bufs=2) as f1p,
            tc.tile_pool(name="f2p", bufs=3) as f2p,
            tc.tile_pool(name="wk2", bufs=2) as wk2,
            tc.tile_pool(name="ps", bufs=4, space="PSUM") as ps,
            tc.tile_pool(name="psb2", bufs=2, space="PSUM") as psb2,
        ):
            ident = cst.tile([128, 128], F16, tag="ident")
            make_identity(nc, ident[:])
            ones = cst.tile([128, 128], F16, tag="ones")
            nc.gpsimd.memset(ones[:], 1.0)
            eps_t = cst.tile([128, 1], F32, tag="eps")
            nc.gpsimd.memset(eps_t[:], EPS)

            h_tiles = []
            for j in range(NQT):
                ht = hp.tile([128, M], F32, tag=f"h{j}")
                nc.sync.dma_start(out=ht[:], in_=h0[j])
                h_tiles.append(ht)

            enc_t = ep.tile([128, NMC, RW], F16, tag="enc")
            for mc in range(NMC):
                nc.sync.dma_start(out=enc_t[:, mc, :], in_=encT[mc])

            os_t = cst.tile([128, NQT], F32, tag="osc")
            for j in range(NQT):
                nc.sync.dma_start(out=os_t[:, j:j + 1], in_=oscale[j])

            smsk_t = None
            if smsk is not None:
                smsk_t = cst.tile([128, sm_cols], F16, tag="smsk")
                nc.sync.dma_start(out=smsk_t[:], in_=smsk[:])
            cmsk_t = None
            if cmsk is not None:
                cmsk_t = cst.tile([128, cm_cols], F16, tag="cmsk")
                nc.sync.dma_start(out=cmsk_t[:], in_=cmsk[:])

            # ---------------- helpers ----------------
            def layer_norm_T():
                """LN of h (row-major) -> u = (h-mu)*rsig as feature-major
                uT [128(m), NMC, RW] fp16 (gain/bias folded into weights)."""
                uT = utp.tile([128, NMC, RW], F16, tag="uT")
                for j in range(NQT):
                    st = lnp.tile([128, 2, 6], F32, tag="st")
                    nc.vector.bn_stats(st[:, 0, :], h_tiles[j][:, 0:512])
                    nc.vector.bn_stats(st[:, 1, :], h_tiles[j][:, 512:1024])
                    mv = lnp.tile([128, 2], F32, tag="mv")
                    nc.vector.bn_aggr(mv[:], st[:])
                    sd = lnp.tile([128, 1], F32, tag="sd")
                    nc.scalar.activation(sd[:], mv[:, 1:2], AF.Sqrt, bias=eps_t[:])
                    rsig = lnp.tile([128, 1], F32, tag="rsig")
                    nc.vector.reciprocal(rsig[:], sd[:])
                    nmurs = lnp.tile([128, 1], F32, tag="nmurs")
                    nc.vector.tensor_mul(nmurs[:], mv[:, 0:1], rsig[:])
                    nc.scalar.mul(nmurs[:], nmurs[:], -1.0)
                    u = lnp.tile([128, M], F16, tag="u")
                    nc.scalar.activation(u[:], h_tiles[j][:], AF.Identity,
                                         bias=nmurs[:], scale=rsig[:])
                    for mc in range(NMC):
                        tp = ps.tile([128, 128], F16, tag="ps1")
                        nc.tensor.transpose(tp[:], u[:, mc * 128:(mc + 1) * 128],
                                            ident[:])
                        nc.vector.tensor_copy(uT[:, mc, j * 128:(j + 1) * 128],
                                              tp[:])
                return uT

            def kv_phase(l, src, wk_d, wv_d, kb_t, vb_t, ci, co, heads=None):
                """Project K^T/V^T for OWN tiles from src [128, NMC, RW];
                DMA to ci and AllGather (chunked by GH heads) into co."""
                for h in (range(H) if heads is None else heads):
                    wk_t = whp.tile([128, NMC, DK], F16, tag="wkh")
                    wv_t = whp.tile([128, NMC, DK], F16, tag="wvh")
                    nc.sync.dma_start(out=wk_t[:], in_=wk_d[l, h])
                    nc.sync.dma_start(out=wv_t[:], in_=wv_d[l, h])
                    kv_own = kvp.tile([128, 2, RW], F16, tag="kvown")
                    k_ps = ps.tile([128, RW], F32, tag="ps1")
                    for mc in range(NMC):
                        nc.tensor.matmul(k_ps[:], wk_t[:, mc, :], src[:, mc, :],
                                         start=(mc == 0), stop=(mc == NMC - 1))
                    if kb_t is not None:
                        nc.scalar.activation(kv_own[:, 0, :], k_ps[:],
                                             AF.Identity, bias=kb_t[:, h:h + 1])
                    else:
                        nc.scalar.activation(kv_own[:, 0, :], k_ps[:], AF.Identity)
                    v_ps = ps.tile([128, RW], F32, tag="ps1")
                    for mc in range(NMC):
                        nc.tensor.matmul(v_ps[:], wv_t[:, mc, :], src[:, mc, :],
                                         start=(mc == 0), stop=(mc == NMC - 1))
                    vT_tmp = kvp.tile([128, RW], F16, tag="vtt")
                    if vb_t is not None:
                        nc.scalar.activation(vT_tmp[:], v_ps[:],
                                             AF.Identity, bias=vb_t[:, h:h + 1])
                    else:
                        nc.scalar.activation(vT_tmp[:], v_ps[:], AF.Identity)
                    # transpose own V chunks to row-major [t, d] BEFORE the
                    # gather: the transpose work is split across the pair and
                    # the scores phase consumes V directly from the gather
                    for sl in range(NQT):
                        tp = ps.tile([128, 128], F16, tag="ps1")
                        nc.tensor.transpose(
                            tp[:], vT_tmp[:, sl * 128:(sl + 1) * 128], ident[:])
                        nc.vector.tensor_copy(
                            kv_own[:, 1, sl * 128:(sl + 1) * 128], tp[:])
                    nc.sync.dma_start(out=ci[h], in_=kv_own[:])
                    if (h + 1) % GH == 0:
                        g = h // GH
                        if local_cc:
                            for s in range(2):
                                nc.sync.dma_start(
                                    out=co[g, s], in_=ci[g * GH:(g + 1) * GH])
                        else:
                            nc.gpsimd.collective_compute(
                                "AllGather", mybir.AluOpType.bypass,
                                replica_groups=GROUPS,
                                ins=[ci[g * GH:(g + 1) * GH]],
                                outs=[co[g]])

            def q_phase(l, uTq, wq_d, qb_t, tagn):
                q_all = qp.tile([128, H, RW], F16, tag=tagn)
                for h in range(H):
                    wq_t = whp.tile([128, NMC, DK], F16, tag="wqh")
                    nc.sync.dma_start(out=wq_t[:], in_=wq_d[l, h])
                    q_ps = ps.tile([128, RW], F32, tag="ps1")
                    for mc in range(NMC):
                        nc.tensor.matmul(q_ps[:], wq_t[:, mc, :], uTq[:, mc, :],
                                         start=(mc == 0), stop=(mc == NMC - 1))
                    nc.scalar.activation(q_all[:, h, :], q_ps[:], AF.Identity,
                                         bias=qb_t[:, h:h + 1], scale=SC)
                return q_all

            def score_prep(co, suf, h):
                """Load gathered K (feature-major) / V (row-major) for head
                h — no dependency on the q side."""
                g, hh = divmod(h, GH)
                kv_g = kgp.tile([128, 2, 2, RW], F16, tag="kvg")
                for s in range(2):
                    nc.sync.dma_start(out=kv_g[:, s, :, :], in_=co[g, s, hh])
                return kv_g

            def score_phase(q_all, co, suf, regions, msk_t, ptag, preps=None):
                """Per head: suffix-pruned logits -> exp -> den -> att@V ->
                preT, with 1-ahead K/V prep pipelining."""
                active = [t for t in range(NTC) if suf[t] > 0]
                preT = big1.tile([128, H, RW], F16, tag=ptag)
                preps = dict(enumerate(preps or []))
                def ensure(h):
                    if h not in preps:
                        preps[h] = score_prep(co, suf, h)
                for h in range(H):
                    ensure(h)
                    if h + 1 < H:
                        ensure(h + 1)
                    kv_g = preps.pop(h)
                    dp = psb2.tile([128, 2, RW], F32, tag="kv", name="denpre")
                    expT = exp2.tile([128, NTC, RW], F16, tag="expT")
                    for t in active:
                        w = suf[t]
                        lo = RW - w
                        s, sl = OWN[t]
                        lg = ps.tile([128, RW], F32, tag="ps1")
                        nc.tensor.matmul(lg[:, lo:],
                                         kv_g[:, s, 0, sl * 128:(sl + 1) * 128],
                                         q_all[:, h, lo:], start=True, stop=True)
                        for (rtc, qlo, qw, off) in regions:
                            if rtc == t:
                                nc.vector.tensor_add(
                                    lg[:, qlo:qlo + qw], lg[:, qlo:qlo + qw],
                                    msk_t[:, off:off + qw])
                        nc.scalar.activation(expT[:, t, lo:], lg[:, lo:], AF.Exp)
                    for i, t in enumerate(active):
                        lo = RW - suf[t]
                        nc.tensor.matmul(dp[:, 0, lo:], ones[:], expT[:, t, lo:],
                                         start=(i == 0), stop=(i == len(active) - 1),
                                         skip_group_check=True)
                    rden = wk2.tile([128, RW], F32, tag="rden")
                    nc.vector.reciprocal(rden[:], dp[:, 0, :])
                    for i, t in enumerate(active):
                        lo = RW - suf[t]
                        s, sl = OWN[t]
                        nc.tensor.matmul(dp[:, 1, lo:],
                                         kv_g[:, s, 1, sl * 128:(sl + 1) * 128],
                                         expT[:, t, lo:],
                                         start=(i == 0), stop=(i == len(active) - 1),
                                         skip_group_check=True)
                    nc.vector.tensor_mul(preT[:, h, :], dp[:, 1, :], rden[:])
                return preT

            def o_phase(l, preT, wo_d):
                """Output projection + residual (m-half outer, head inner)."""
                wo_all = [wop.tile([128, H, 512], F16, tag=f"woall{mh}",
                                   name=f"woall{mh}")
                          for mh in range(2)]
                for mh in range(2):
                    nc.sync.dma_start(out=wo_all[mh][:], in_=wo_d[l, mh])
                for mh in range(2):
                    ms = slice(mh * 512, (mh + 1) * 512)
                    o_half = [psb2.tile([128, 2, 512], F32, tag="kv",
                                        name=f"oh{jh}") for jh in range(2)]
                    for h in range(H):
                        for j in range(NQT):
                            nc.tensor.matmul(o_half[j // 2][:, j % 2, :],
                                             preT[:, h, j * 128:(j + 1) * 128],
                                             wo_all[mh][:, h, :],
                                             start=(h == 0), stop=(h == H - 1),
                                             skip_group_check=True)
                    for j in range(NQT):
                        o_sb = wk2.tile([128, 512], F32, tag="osb")
                        nc.scalar.activation(o_sb[:], o_half[j // 2][:, j % 2, :],
                                             AF.Identity, scale=os_t[:, j:j + 1])
                        nc.vector.tensor_add(h_tiles[j][:, ms],
                                             h_tiles[j][:, ms], o_sb[:])

            def ffn_phase(l):
                fb1_t = wk2.tile([128, NFC], F32, tag="fb1")
                nc.sync.dma_start(out=fb1_t[:], in_=fb1[l])
                fb2_t = wk2.tile([1, M], F16, tag="fb2")
                nc.sync.dma_start(out=fb2_t[:], in_=fb2[l])
                u3T = layer_norm_T()
                sT = big1.tile([128, NFC, RW], F16, tag="sT")
                for qf in range(8):          # eighths of F
                    fw1_t = f1p.tile([128, NMC, 512], F16, tag="fw1")
                    nc.sync.dma_start(out=fw1_t[:], in_=fw1[l, qf])
                    for fcl in range(4):
                        fc = qf * 4 + fcl
                        s_ps = ps.tile([128, RW], F32, tag="ps1")
                        for mc in range(NMC):
                            nc.tensor.matmul(
                                s_ps[:],
                                fw1_t[:, mc, fcl * 128:(fcl + 1) * 128],
                                u3T[:, mc, :],
                                start=(mc == 0), stop=(mc == NMC - 1))
                        nc.scalar.activation(sT[:, fc, :], s_ps[:], AF.Relu,
                                             bias=fb1_t[:, fc:fc + 1])
                for mh in range(2):
                    ms = slice(mh * 512, (mh + 1) * 512)
                    f_half = [psb2.tile([128, 2, 512], F32, tag="kv",
                                        name=f"fh{jh}") for jh in range(2)]
                    for fg in range(8):
                        fw2_t = f2p.tile([128, 4, 512], F16, tag="fw2")
                        nc.sync.dma_start(out=fw2_t[:], in_=fw2[l, mh, fg])
                        for fi in range(4):
                            fc = fg * 4 + fi
                            for j in range(NQT):
                                nc.tensor.matmul(f_half[j // 2][:, j % 2, :],
                                                 sT[:, fc, j * 128:(j + 1) * 128],
                                                 fw2_t[:, fi, :],
                                                 start=(fc == 0), stop=False,
                                                 skip_group_check=True)
                    for j in range(NQT):
                        nc.tensor.matmul(f_half[j // 2][:, j % 2, :], ones[0:1, :],
                                         fb2_t[0:1, ms], start=False, stop=True,
                                         skip_group_check=True)
                        f_sb = wk2.tile([128, 512], F32, tag="fsb")
                        nc.scalar.activation(f_sb[:], f_half[j // 2][:, j % 2, :],
                                             AF.Identity, scale=os_t[:, j:j + 1])
                        nc.vector.tensor_add(h_tiles[j][:, ms],
                                             h_tiles[j][:, ms], f_sb[:])

            # ---------------- the layers ----------------
            for l in range(nlayers):
                qb1_t = wk2.tile([128, H], F32, tag="qb1t")
                nc.sync.dma_start(out=qb1_t[:], in_=qb1[l])
                kb1_t = wk2.tile([128, H], F32, tag="kb1t")
                nc.sync.dma_start(out=kb1_t[:], in_=kb1[l])
                vb1_t = wk2.tile([128, H], F32, tag="vb1t")
                nc.sync.dma_start(out=vb1_t[:], in_=vb1[l])
                qb2_t = wk2.tile([128, H], F32, tag="qb2t")
                nc.sync.dma_start(out=qb2_t[:], in_=qb2[l])

                # cross K/V needs only enc + weights: schedule its head
                # groups to give the PE ready work during the LN stalls
                # (heads 0-3 of layer l ran before LN3 of layer l-1).
                if l == 0:
                    kv_phase(l, enc_t, ckw, cvw, None, None, cic[l], coc[l],
                             heads=range(0, GH))
                kv_phase(l, enc_t, ckw, cvw, None, None, cic[l], coc[l],
                         heads=range(GH, H))
                u1T = layer_norm_T()
                kv_phase(l, u1T, skw, svw, kb1_t, vb1_t, cis[l], cos[l])
                q1 = q_phase(l, u1T, sqw, qb1_t, "qall")

                if 'self' not in ABLATE:
                    preT = score_phase(q1, cos[l], s_suf, s_regions, smsk_t,
                                       "preT")
                    o_phase(l, preT, swo)

                cpre = None
                if 'cross' not in ABLATE:
                    cpre = [score_prep(coc[l], c_suf, 0),
                            score_prep(coc[l], c_suf, 1)]
                # next layer's cross-KV, split so each LN stall has ready
                # PE work: heads 0-1 cover LN2, 2-3 cover LN3 (the gather
                # for group 0 fires at head 3), 4-7 cover LN1 next layer
                if l + 1 < nlayers:
                    kv_phase(l + 1, enc_t, ckw, cvw, None, None,
                             cic[l + 1], coc[l + 1], heads=range(0, 2))
                u2T = layer_norm_T()
                if 'cross' not in ABLATE:
                    q2 = q_phase(l, u2T, cqw, qb2_t, "qall")
                    preT2 = score_phase(q2, coc[l], c_suf, c_regions, cmsk_t,
                                        "preT", preps=cpre)
                    o_phase(l, preT2, cwo)

                if l + 1 < nlayers:
                    kv_phase(l + 1, enc_t, ckw, cvw, None, None,
                             cic[l + 1], coc[l + 1], heads=range(2, GH))
                if 'ffn' in ABLATE:
                    layer_norm_T()
                    continue
                ffn_phase(l)

            for j in range(NQT):
                nc.sync.dma_start(out=hout[j], in_=h_tiles[j][:])

    nc.compile()
    return nc


# --------------------------------------------------------------------------
# host side
# --------------------------------------------------------------------------

def _prep_weights(lo, hi, swq, swk, swv, swo_, cwq, cwk, cwv, cwo_,
                  w1, b1, w2, b2, ln1_g, ln1_b, ln2_g, ln2_b, ln3_g, ln3_b):
    """Fold LN gains/biases into weights; reshape for tile-friendly DMA."""
    d = {}
    nl = hi - lo
    sl = slice(lo, hi)

    def proj_fold(w, g):   # [nl,H,M,DK] * g[nl,M] -> [nl,H,128,NMC,DK] fp16
        wf = w * g[:, None, :, None]
        return np.ascontiguousarray(
            wf.reshape(nl, H, NMC, 128, DK).transpose(0, 1, 3, 2, 4)
        ).astype(np.float16)

    def proj_bias(w, b):   # -> [nl,128(d),H]
        bb = np.einsum('lhmd,lm->lhd', w, b)
        return np.ascontiguousarray(bb.transpose(0, 2, 1)).astype(np.float32)

    d['sqw'] = proj_fold(swq[sl], ln1_g[sl])
    d['skw'] = proj_fold(swk[sl], ln1_g[sl])
    d['svw'] = proj_fold(swv[sl], ln1_g[sl])
    d['qb1'] = proj_bias(swq[sl], ln1_b[sl]) * SC
    d['kb1'] = proj_bias(swk[sl], ln1_b[sl])
    d['vb1'] = proj_bias(swv[sl], ln1_b[sl])
    d['swo'] = np.ascontiguousarray(
        swo_[sl].reshape(nl, H, 128, 2, 512).transpose(0, 3, 2, 1, 4)
    ).astype(np.float16)

    ones_g = np.ones((nl, M), swq.dtype)
    d['cqw'] = proj_fold(cwq[sl], ln2_g[sl])
    d['qb2'] = proj_bias(cwq[sl], ln2_b[sl]) * SC
    d['ckw'] = proj_fold(cwk[sl], ones_g)
    d['cvw'] = proj_fold(cwv[sl], ones_g)
    d['cwo'] = np.ascontiguousarray(
        cwo_[sl].reshape(nl, H, 128, 2, 512).transpose(0, 3, 2, 1, 4)
    ).astype(np.float16)

    fw1 = w1[sl] * ln3_g[sl][:, :, None]
    d['fw1'] = np.ascontiguousarray(
        fw1.reshape(nl, NMC, 128, 8, 512).transpose(0, 3, 2, 1, 4)
    ).astype(np.float16)
    fb1 = b1[sl] + np.einsum('lmf,lm->lf', w1[sl], ln3_b[sl])
    d['fb1'] = np.ascontiguousarray(
        fb1.reshape(nl, NFC, 128).transpose(0, 2, 1)).astype(np.float32)
    d['fw2'] = np.ascontiguousarray(
        w2[sl].reshape(nl, 8, 4, 128, 2, 512).transpose(0, 4, 1, 3, 2, 5)
    ).astype(np.float16)
    d['fb2'] = b2[sl].reshape(nl, 1, M).astype(np.float16)
    return d


def _tiles(core):
    return TILES_EVEN if core % 2 == 0 else TILES_ODD


def _rows(tiles):
    return np.concatenate([np.arange(t * 128, (t + 1) * 128) for t in tiles])


def _plan(maskT_all):
    """maskT_all: per-core [T, RW] masks (local q cols in slot order,
    values in [MASK_CLIP, 0]).  Returns (suf, regions, packed) with
    uniform suffix widths and mask-add regions."""
    full_masked = lambda blk: np.all(blk <= MASK_CLIP * 0.999)
    suf = []
    for t in range(NTC):
        w = 0
        for mt in maskT_all:
            for sl in range(NQT):
                blk = mt[t * 128:(t + 1) * 128, sl * 128:(sl + 1) * 128]
                if not full_masked(blk):
                    w = max(w, RW - sl * 128)
                    break
        suf.append(w)
    if suf[0] == 0:
        suf[0] = RW   # keep start=True covering all columns
    else:
        suf[0] = RW
    regions = []
    off = 0
    for t in range(NTC):
        if suf[t] == 0:
            continue
        lo_b = RW - suf[t]
        nz = np.zeros(RW, bool)
        for mt in maskT_all:
            nz |= np.any(mt[t * 128:(t + 1) * 128, :] != 0, axis=0)
        nz[:lo_b] = False    # outside computed suffix: never applied
        idx = np.nonzero(nz)[0]
        if len(idx) == 0:
            continue
        lo, hi = int(idx[0]), int(idx[-1]) + 1
        regions.append((t, lo, hi - lo, off))
        off += hi - lo
    cols = off
    packed = None
    if cols:
        packed = []
        for mt in maskT_all:
            pk = np.zeros((128, cols), np.float16)
            for (t, lo, w, o) in regions:
                pk[:, o:o + w] = mt[t * 128:(t + 1) * 128, lo:lo + w]
            packed.append(pk)
    return suf, regions, packed


def build_noop(nlayers, s_suf, s_regions, c_suf, c_regions, sm_cols, cm_cols):
    """Same I/O signature as build_decoder but only copies h0 -> hout.
    Used to measure the fixed dispatch/transfer overhead of a run."""
    nc = bacc.Bacc(None)
    names = dict(h0=[NQT, 128, M], encT=[NMC, 128, RW], oscale=[NQT, 128, 1],
                 sqw=[nlayers, H, 128, NMC, DK], skw=[nlayers, H, 128, NMC, DK],
                 svw=[nlayers, H, 128, NMC, DK], swo=[nlayers, 2, 128, H, 512],
                 qb1=[nlayers, 128, H], kb1=[nlayers, 128, H],
                 vb1=[nlayers, 128, H],
                 cqw=[nlayers, H, 128, NMC, DK], ckw=[nlayers, H, 128, NMC, DK],
                 cvw=[nlayers, H, 128, NMC, DK], cwo=[nlayers, 2, 128, H, 512],
                 qb2=[nlayers, 128, H],
                 fw1=[nlayers, 8, 128, NMC, 512], fb1=[nlayers, 128, NFC],
                 fw2=[nlayers, 2, 8, 128, 4, 512], fb2=[nlayers, 1, M])
    if sm_cols:
        names['smsk'] = [128, sm_cols]
    if cm_cols:
        names['cmsk'] = [128, cm_cols]
    f32set = {'h0', 'oscale', 'qb1', 'kb1', 'vb1', 'qb2', 'fb1'}
    ts = {}
    for nm, shp in names.items():
        ts[nm] = nc.dram_tensor(nm, shp, F32 if nm in f32set else F16,
                                kind="ExternalInput")
    hout = nc.dram_tensor("hout", [NQT, 128, M], F32, kind="ExternalOutput")
    with TileContext(nc) as tc:
        with tc.tile_pool(name="sb", bufs=2) as sb:
            for j in range(NQT):
                t = sb.tile([128, M], F32, tag="t")
                nc.sync.dma_start(out=t[:], in_=ts['h0'][j])
                nc.sync.dma_start(out=hout[j], in_=t[:])
    nc.compile()
    return nc


CHUNK = 6   # layers per device program; 6 = whole decoder in one exec


class _Runner:
    """Persistent PJRT execution handle for one compiled Bass program.

    run_bass_kernel_spmd re-traces/re-jits on every call and re-ships every
    input from host; here the shard_map(bass_exec) wrapper is jitted once and
    all inputs live on device, so the per-call path is pure dispatch.  The
    zero-init output "seed" buffers are passed un-donated (both programs
    write every element of every output), so one persistent set suffices.
    """

    def __init__(self, nc, n_cores=NCORE):
        import jax
        from jax.experimental.shard_map import shard_map
        from jax.sharding import Mesh, NamedSharding, PartitionSpec
        from concourse import bass2jax

        bass2jax.install_neuronx_cc_hook()
        self._jax = jax
        self.nc = nc
        self.n_cores = n_cores

        partition_name = (nc.partition_id_tensor.name
                          if nc.partition_id_tensor else None)
        self.dbg_name = None
        if nc.dbg_addr is not None:
            assert not nc.dbg_callbacks, "dbg callbacks unsupported here"
            self.dbg_name = nc.dbg_addr.name
        in_names, out_names, out_avals, out_shapes = [], [], [], []
        for alloc in nc.m.functions[0].allocations:
            if not isinstance(alloc, mybir.MemoryLocationSet):
                continue
            name = alloc.memorylocations[0].name
            if alloc.kind == "ExternalInput":
                if name != partition_name:
                    in_names.append(name)
            elif alloc.kind == "ExternalOutput":
                shape = tuple(alloc.tensor_shape)
                dtype = mybir.dt.np(alloc.dtype)
                out_names.append(name)
                out_avals.append(jax.core.ShapedArray(shape, dtype))
                out_shapes.append((shape, dtype))
        self.in_names = list(in_names)
        self.out_names = list(out_names)
        ext_names = in_names + out_names
        if partition_name is not None:
            ext_names.append(partition_name)
        n_params, n_outs = len(in_names), len(out_names)

        def _body(*args):
            operands = list(args)
            if partition_name is not None:
                operands.append(bass2jax.partition_id_tensor())
            outs = bass2jax._bass_exec_p.bind(
                *operands,
                out_avals=tuple(out_avals),
                in_names=tuple(ext_names),
                out_names=tuple(out_names),
                lowering_input_output_aliases=(),
                sim_require_finite=True,
                sim_require_nnan=True,
                nc=nc,
            )
            return tuple(outs)

        devices = jax.devices()[:n_cores]
        assert len(devices) == n_cores, (len(jax.devices()), n_cores)
        mesh = Mesh(np.asarray(devices), ("core",))
        self.sharding = NamedSharding(mesh, PartitionSpec("core"))
        in_specs = (PartitionSpec("core"),) * (n_params + n_outs)
        out_specs = (PartitionSpec("core"),) * n_outs
        self.fn = jax.jit(
            shard_map(_body, mesh=mesh, in_specs=in_specs,
                      out_specs=out_specs, check_rep=False),
            keep_unused=True,
        )
        self.out_seed = [
            jax.device_put(np.zeros((n_cores * s[0], *s[1:]), d),
                           self.sharding)
            for (s, d) in out_shapes
        ]
        self._staged = {}

    def stage(self, name, arrs):
        """Per-core host arrays -> one global device-resident array.
        Dedupes on object identity so shared arrays upload once."""
        if name == self.dbg_name:
            arrs = [np.zeros((1, 2), np.uint32)] * self.n_cores
        key = (name,) + tuple(id(a) for a in arrs)
        hit = self._staged.get(key)
        if hit is None:
            glob = np.concatenate([np.ascontiguousarray(a) for a in arrs], 0)
            dev = self._jax.device_put(glob, self.sharding)
            self._staged[key] = hit = (dev, arrs)  # arrs pins the id()s
        return hit[0]

    def call(self, args):
        return self.fn(*args, *self.out_seed)


class RunHandle:
    def __init__(self, runner, chunk_args, h0_pos, hout_pos):
        self.runner = runner
        self.chunk_args = chunk_args
        self.h0_pos = h0_pos
        self.hout_pos = hout_pos


def prepare(inputs, nlayers=L, noop=False):
    """Builds (or reuses) the device program, stages every chunk's inputs on
    device, and returns a RunHandle whose per-run cost is dispatch only."""
    enc_out = np.asarray(inputs['enc_out'])
    x = np.asarray(inputs['x'])
    position_mask = np.asarray(inputs['position_mask'])
    qt_self_mask = np.asarray(inputs['qt_self_mask'])
    qt_cross_mask = np.asarray(inputs['qt_cross_mask'])

    smT, cmT = [], []
    for c in range(NCORE):
        b = c // 2
        rows = _rows(_tiles(c))
        sm = np.maximum(position_mask[b][None, :], qt_self_mask[b])  # [Q,T]
        sm = np.clip(sm[rows].T * (-1e6 * SC), MASK_CLIP, 0).astype(np.float32)
        cm = np.clip(qt_cross_mask[b][rows].T * (-1e6 * SC),
                     MASK_CLIP, 0).astype(np.float32)
        smT.append(sm)
        cmT.append(cm)
    s_suf, s_regions, smsk_packed = _plan(smT)
    c_suf, c_regions, cmsk_packed = _plan(cmT)
    sm_cols = sum(r[2] for r in s_regions)
    cm_cols = sum(r[2] for r in c_regions)

    chunk = min(CHUNK, nlayers)
    nchunks = (nlayers + chunk - 1) // chunk
    assert nlayers == chunk * nchunks, (nlayers, chunk)

    key = (chunk, noop, tuple(s_suf), tuple(s_regions),
           tuple(c_suf), tuple(c_regions))
    if key not in _CACHE:
        builder = build_noop if noop else build_decoder
        nc = builder(chunk, s_suf, s_regions, c_suf, c_regions,
                     sm_cols, cm_cols)
        _CACHE[key] = (nc, _Runner(nc))
    nc, runner = _CACHE[key]

    warrs = [np.asarray(inputs[k]) for k in
             ('swq', 'swk', 'swv', 'swo', 'cwq', 'cwk', 'cwv', 'cwo',
              'w1', 'b1', 'w2', 'b2', 'ln1_g', 'ln1_b', 'ln2_g', 'ln2_b',
              'ln3_g', 'ln3_b')]

    # per-core constants shared by every chunk (hoisted so staging dedupes)
    enc_c, osc_c, h0_c = [], [], []
    for c in range(NCORE):
        b = c // 2
        tiles = _tiles(c)
        rows = _rows(tiles)
        encT_full = enc_out[b].T.reshape(NMC, 128, T)
        enc_c.append(np.ascontiguousarray(
            encT_full[:, :, rows]).astype(np.float16))
        osc_c.append(np.ascontiguousarray(
            (1.0 - position_mask[b, rows]).reshape(NQT, 128, 1)
        ).astype(np.float32))
        h0_c.append(np.ascontiguousarray(
            x[b][rows].reshape(NQT, 128, M)).astype(np.float32))

    chunk_args = []
    for k in range(nchunks):
        wd = _prep_weights(k * chunk, (k + 1) * chunk, *warrs)
        args = []
        for name in runner.in_names:
            if name == 'h0':
                arrs = h0_c           # placeholder for k>0 (swapped at run)
            elif name == 'encT':
                arrs = enc_c
            elif name == 'oscale':
                arrs = osc_c
            elif name == 'smsk':
                arrs = smsk_packed
            elif name == 'cmsk':
                arrs = cmsk_packed
            elif name == runner.dbg_name:
                arrs = None
            else:
                arrs = [wd[name]] * NCORE
            args.append(runner.stage(name, arrs))
        chunk_args.append(args)

    return RunHandle(runner, chunk_args,
                     runner.in_names.index('h0'),
                     runner.out_names.index('hout'))


def forward(handle):
    """Dispatch one full forward (all chunks, h chained on device); returns
    the final global hout device array without blocking."""
    r = handle.runner
    h = None
    for args in handle.chunk_args:
        if h is not None:
            args = list(args)
            args[handle.h0_pos] = h
        outs = r.call(args)
        h = outs[handle.hout_pos]
    return h


def run(handle):
    hout = np.asarray(forward(handle))
    hout = hout.reshape(NCORE, NQT, 128, M)
    out = np.empty((B, Q, M), np.float32)
    for c in range(NCORE):
        b = c // 2
        for sl, g in enumerate(_tiles(c)):
            out[b, g * 128:(g + 1) * 128] = hout[c, sl]
    return out


def kernel(enc_out, x, position_mask, qt_self_mask, qt_cross_mask,
           swq, swk, swv, swo, cwq, cwk, cwv, cwo,
           w1, b1, w2, b2, ln1_g, ln1_b, ln2_g, ln2_b, ln3_g, ln3_b,
           nlayers=L):
    inputs = dict(enc_out=enc_out, x=x, position_mask=position_mask,
                  qt_self_mask=qt_self_mask, qt_cross_mask=qt_cross_mask,
                  swq=swq, swk=swk, swv=swv, swo=swo,
                  cwq=cwq, cwk=cwk, cwv=cwv, cwo=cwo,
                  w1=w1, b1=b1, w2=w2, b2=b2,
                  ln1_g=ln1_g, ln1_b=ln1_b, ln2_g=ln2_g, ln2_b=ln2_b,
                  ln3_g=ln3_g, ln3_b=ln3_b)
    handle = prepare(inputs, nlayers=nlayers)
    return run(handle)
